# revision 71
# baseline (speedup 1.0000x reference)
"""Trainium2 Bass kernel for the attention module (b=4, c=256, l=2048, h=8, d=64).

Sharding: 8 cores = 4 batches x 2 query-halves (no collectives). Each core
receives its batch's x with columns permuted so its own query half comes
first; it computes k/v for all 2048 key positions and the attention output
for its 1024 queries, then the output projection + final rms-norm.

Device algorithm per core (all matmul data bf16, psum fp32):
  ss_j   = sum_c bf16(x[c,j]^2)              (PE ones-reduce)
  s_j    = 1/sqrt(ss_j/256 + eps)            (ACT sqrt + DVE reciprocal)
  xn     = bf16(x * bcast(s))                (PE ones-broadcast + DVE)
  q,k    = bf16(wqkvT-slices.T @ xn)         (g1 folded on host)
  vT     = bf16(xn_slice.T @ wv), ones col fused at index 64
  per head:  simT[j,i] = k_h.T @ q_h         (psum fp32, logits*8)
             pt = exp(simT/8)  as bf16       (split ~9/7 between ACT true
                                              exp and DVE Schraudolph
                                              bitcast-exp via uint16)
             ytps[i, d|den] += pt_chunk.T @ [vT_h|1]   (transposed y, fused den)
  y_h    = ytps[:, 0:64] * (1/den) broadcast (DVE, stride-0 bcast)
  y4     = PE transpose(yts, identity)       (bf16, per head-pair)
  out    = woT.T @ y4 + b_out
  result = out * bcast(1/sqrt(ss2/256+eps)) * g2

Scheduling: one rotating 3x[128,1024] psum arena serves the norm, all
projections, sim chunks, the pair transposes, and the out-proj/final-norm
tail, so no phase blocks on a psum-bank WAR against another pool. Head h's
sim+exp stream overlaps head h-1's yT accumulation (whose pt is fully
materialized, so yT never waits on an exp producer). exp jts interleave
across ACT/DVE so both engines drain the triple-buffered ring in parallel.
Late-consumed tensors (idn/wo/bo/g2) are DMA'd on the gpsimd SWDGE queue
because a queue's waiters effectively wait on the whole queue's clock.
"""
import sys

import numpy as np

if "/opt/trn_rl_repo" not in sys.path:
    sys.path.insert(0, "/opt/trn_rl_repo")

import ml_dtypes  # noqa: E402

import concourse.bass as bass  # noqa: E402
import concourse.tile as tile  # noqa: E402
from concourse import bacc, mybir  # noqa: E402
from concourse.bass_utils import run_bass_kernel_spmd  # noqa: E402

F32 = mybir.dt.float32
F32R = mybir.dt.float32r
BF16 = mybir.dt.bfloat16
FP8 = mybir.dt.float8e4
U16 = mybir.dt.uint16
AF = mybir.ActivationFunctionType
DR = mybir.MatmulPerfMode.DoubleRow
MUL = mybir.AluOpType.mult
ADD = mybir.AluOpType.add

B, C, L = 4, 256, 2048
H, D = 8, 64
HID = H * D
LQ = L // 2      # queries per core
NCT = C // 128   # 2 c-tiles
NJ = L // 128    # 16 j-tiles
NI = LQ // 128   # 8 i-chunks
EPS_B = 1e-26

LOG2E = 1.4426950408889634
# Schraudolph bf16: bits = round(logit*128*log2e + 127*128 - 5.6); logit = sim/8
SCH_A = 128.0 * LOG2E / 8.0
SCH_B = 127.0 * 128.0 - 5.6

# exp engine split: jts are interleaved across ACT (true exp) and DVE
# (Schraudolph bitcast-exp) so both engines drain the sim psum in parallel;
# any 3 consecutive jts touch both engines (sim psum is triple-buffered).
# gpsimd cannot read PSUM on TRN2, so it gets no exp work.
DVE_JT = frozenset((1, 3, 5, 7, 9, 12, 14))


def _body(tc, x, wq, wo, bo, g2, idn, out):
    nc = tc.nc
    from contextlib import ExitStack
    with ExitStack() as ctx:
        ctx.enter_context(nc.allow_low_precision(
            reason="bf16/fp8 data path by design"))
        const = ctx.enter_context(tc.tile_pool(name="const", bufs=1))
        big = ctx.enter_context(tc.tile_pool(name="big", bufs=2))
        xnp = ctx.enter_context(tc.tile_pool(name="xn", bufs=4))
        sqp = ctx.enter_context(tc.tile_pool(name="sq", bufs=1))
        qp = ctx.enter_context(tc.tile_pool(name="q", bufs=4))
        kp = ctx.enter_context(tc.tile_pool(name="k", bufs=4))
        vtp = ctx.enter_context(tc.tile_pool(name="vt", bufs=16))
        ptp = ctx.enter_context(tc.tile_pool(name="pt", bufs=2))
        invp = ctx.enter_context(tc.tile_pool(name="inv", bufs=2))
        ytsp = ctx.enter_context(tc.tile_pool(name="yts", bufs=2))
        y4p = ctx.enter_context(tc.tile_pool(name="y4", bufs=4))
        widep = ctx.enter_context(tc.tile_pool(name="wide", bufs=2))
        outp = ctx.enter_context(tc.tile_pool(name="outp", bufs=2))
        finp = ctx.enter_context(tc.tile_pool(name="fin", bufs=2))

        # ---------------- constants & weights ----------------
        stage = const.tile([128, 8], F32, tag="stage")
        nc.vector.memset(stage[:], 1.0)
        ones_col = const.tile([128, 1], BF16, tag="ones_col")
        nc.vector.tensor_copy(ones_col[:], stage[:, 0:1])
        ones_row = const.tile([1, 128], F32R, tag="ones_row")
        nc.vector.tensor_copy(ones_row[:],
                              stage[0:1, 0:1].broadcast_to((1, 128)))
        eps_t = const.tile([1, 1], F32, tag="eps")
        nc.vector.memset(eps_t[:], EPS_B)
        zero_t = const.tile([128, 1], F32, tag="zero")
        nc.vector.memset(zero_t[:], 0.0)

        # Critical-path DMAs (x, wq) ride the ACT HWDGE queue alone: waiters
        # of a queue's completion sem effectively wait for the whole queue,
        # so late-consumed tensors go on SP instead.
        x_sb = []
        for ct in range(NCT):
            t = big.tile([128, L], BF16, tag="big")
            x_sb.append(t)
        for half in range(2):
            for ct in range(NCT):
                nc.sync.dma_start(
                    x_sb[ct][:, half * 1024:(half + 1) * 1024],
                    x[ct * 128:(ct + 1) * 128,
                      half * 1024:(half + 1) * 1024])

        wq_sb = []
        for ct in range(NCT):
            t = const.tile([128, 3 * HID], BF16, tag=f"wq{ct}")
            nc.sync.dma_start(t[:], wq[ct * 128:(ct + 1) * 128, :])
            wq_sb.append(t)
        # idn/wo/bo/g2 are needed only late; their dma_starts are emitted
        # after the projections so early consumers don't wait on them (DMA
        # completion sems are threshold-counted per queue).
        idn_sb = const.tile([128, 128], BF16, tag="idn")
        wo_sb = [const.tile([128, C], BF16, tag=f"wo{kt}", name=f"wo{kt}")
                 for kt in range(4)]
        bo_sb = [const.tile([128, 1], F32, tag=f"bo{mt}", name=f"bo{mt}")
                 for mt in range(2)]
        g2_sb = [const.tile([128, 1], F32, tag=f"g2{mt}", name=f"g2{mt}")
                 for mt in range(2)]

        def _late_dmas():
            # Pool SWDGE: keeps the SP HWDGE queue (x, wq, out) clean so
            # early consumers' coalesced DMA-clock waits release early.
            nc.gpsimd.dma_start(idn_sb[:], idn)
            for kt in range(4):
                nc.gpsimd.dma_start(wo_sb[kt][:],
                                    wo[kt * 128:(kt + 1) * 128, :])
            for mt in range(2):
                nc.gpsimd.dma_start(bo_sb[mt][:],
                                    bo[mt * 128:(mt + 1) * 128, :])
                nc.gpsimd.dma_start(g2_sb[mt][:],
                                    g2[mt * 128:(mt + 1) * 128, :])

        # ------------- unified psum pipeline: norm + proj + attention ------
        # ONE rotating psum arena (3 x [128,1024]) serves the norm's ss/bc,
        # all projections, and the sim chunks, so no phase ever blocks on a
        # psum-bank WAR against a different pool's slowest reader.
        xn_sb = [[None] * 2 for _ in range(NCT)]
        q_sb, k_sb = [None] * 4, [None] * 4
        vt_sb = [None] * NJ
        y4_sb = []
        with tc.tile_pool(name="ps", bufs=3, space="PSUM") as psp, \
                tc.tile_pool(name="ps_yt", bufs=2, space="PSUM") as ps_yt:

            def pstile(name):
                return psp.tile([128, 1024], F32, tag="sim", name=name)

            # --- input rms-norm: per-engine stage batching so the DVE
            # queue never buries a later half behind an earlier half ---
            s1 = widep.tile([1, L], F32R, tag="wide")
            sq8t, sst, stmpt, bct = [], [], [], []
            for half in range(2):
                hs = slice(half * 1024, (half + 1) * 1024)
                t = sqp.tile([128, 2, 1024], BF16, tag="sq8",
                             name=f"sq8_{half}")
                for ct in range(NCT):
                    nc.vector.tensor_mul(t[:, ct, :],
                                         x_sb[ct][:, hs], x_sb[ct][:, hs])
                sq8t.append(t)
            for half in range(2):
                sst.append(pstile(f"ssps{half}"))
                for n in range(2):
                    for ct in range(NCT):
                        nc.tensor.matmul(
                            sst[half][0:1, n * 512:(n + 1) * 512],
                            ones_col[:],
                            sq8t[half][:, ct, n * 512:(n + 1) * 512],
                            start=(ct == 0), stop=(ct == NCT - 1))
            for half in range(2):
                stmp = widep.tile([1, 1024], F32, tag="stmp",
                                  name=f"stmp{half}")
                nc.scalar.activation(stmp[:], sst[half][0:1, :], AF.Sqrt,
                                     bias=eps_t[:], scale=1.0 / C)
                stmpt.append(stmp)
            for half in range(2):
                hs = slice(half * 1024, (half + 1) * 1024)
                nc.vector.reciprocal(s1[0:1, hs], stmpt[half][:])
            for half in range(2):
                bct.append(pstile(f"bcps{half}"))
                for n in range(2):
                    cs = slice(half * 1024 + n * 512,
                               half * 1024 + (n + 1) * 512)
                    nc.tensor.matmul(bct[half][:, n * 512:(n + 1) * 512],
                                     ones_row[:], s1[0:1, cs],
                                     start=True, stop=True)
            for half in range(2):
                hs = slice(half * 1024, (half + 1) * 1024)
                for ct in range(NCT):
                    t = xnp.tile([128, 1024], BF16, tag="xn",
                                 name=f"xn{ct}_{half}")
                    nc.vector.tensor_mul(t[:], x_sb[ct][:, hs],
                                         bct[half][:, :])
                    xn_sb[ct][half] = t

            # --- projection helpers (psum from the shared arena) ---
            def make_q(mt):
                ps = pstile(f"qps{mt}")
                for n in range(LQ // 512):
                    for ct in range(NCT):
                        nc.tensor.matmul(
                            ps[:, n * 512:(n + 1) * 512],
                            wq_sb[ct][:, mt * 128:(mt + 1) * 128],
                            xn_sb[ct][0][:, n * 512:(n + 1) * 512],
                            start=(ct == 0), stop=(ct == NCT - 1))
                t = qp.tile([128, LQ], BF16, tag="q", name=f"qsb{mt}")
                nc.scalar.copy(t[:], ps[:, :])
                q_sb[mt] = t

            def make_k_half(mt, half, eng):
                if half == 0:
                    k_sb[mt] = kp.tile([128, L], BF16, tag="k",
                                       name=f"ksb{mt}")
                t = k_sb[mt]
                ps = pstile(f"kps{mt}_{half}")
                for n in range(2):
                    for ct in range(NCT):
                        nc.tensor.matmul(
                            ps[:, n * 512:(n + 1) * 512],
                            wq_sb[ct][:, HID + mt * 128:HID + (mt + 1) * 128],
                            xn_sb[ct][half][:, n * 512:(n + 1) * 512],
                            start=(ct == 0), stop=(ct == NCT - 1))
                if eng is nc.scalar:
                    eng.copy(t[:, half * LQ:(half + 1) * LQ], ps[:, :])
                else:
                    eng.tensor_copy(t[:, half * LQ:(half + 1) * LQ], ps[:, :])

            def make_vt(jt):
                ps = pstile(f"vps{jt}")
                for ct in range(NCT):
                    nc.tensor.matmul(ps[:, 0:512],
                                     xn_sb[ct][jt // 8][:, (jt % 8) * 128:
                                                        (jt % 8 + 1) * 128],
                                     wq_sb[ct][:, 2 * HID:3 * HID],
                                     start=(ct == 0), stop=(ct == NCT - 1))
                t = vtp.tile([128, H, D + 1], BF16, tag="vt", name=f"vt{jt}")
                src = ps[:, 0:512].rearrange("p (h e) -> p h e", e=D)
                if jt % 2 == 0:
                    nc.vector.tensor_copy(t[:, :, 0:D], src)
                else:
                    nc.scalar.copy(t[:, :, 0:D], src)
                nc.gpsimd.tensor_copy(
                    t[:, :, D:D + 1].rearrange("p h o -> p (h o)"),
                    stage[:, 0:H])
                vt_sb[jt] = t

            make_q(0)
            make_k_half(0, 0, nc.vector)
            make_k_half(0, 1, nc.scalar)
            _late_dmas()
            for jt in range(8):
                make_vt(jt)

            # --- attention: head h sims + exp stream while head h-1's yT
            # accumulates from its fully-materialized pt tile; projections
            # for later heads interleave into h0-h3's slots. ---
            pt_sb = [None] * H
            yts_sb = [None] * 4
            out_ps = []
            state = {}

            def sim_chunk(h, jt):
                mt, po = h // 2, (h % 2) * D
                sps = pstile(f"sps{h}_{jt}")
                for n in range(LQ // 512):
                    nc.tensor.matmul(
                        sps[:, n * 512:(n + 1) * 512],
                        k_sb[mt][po:po + D, jt * 128:(jt + 1) * 128],
                        q_sb[mt][po:po + D, n * 512:(n + 1) * 512],
                        start=True, stop=True)
                dst = pt_sb[h][:, jt, :]
                if jt in DVE_JT:
                    nc.vector.tensor_scalar(
                        dst.bitcast(U16), sps[:, :], SCH_A, SCH_B,
                        op0=MUL, op1=ADD)
                else:
                    nc.scalar.activation(dst, sps[:, :], AF.Exp,
                                         bias=zero_t[:], scale=0.125)

            def yt_start(h):
                state["yps"] = ps_yt.tile([128, 4, 128], F32, tag="yt",
                                          name=f"yt{h}_0")

            def yt_step(h, step):
                """steps 0..31: (half, ic, jts 0..15) in 8-instr groups.
                step s covers half=s//16, ic=(s//4)%4, jts 4*(s%4)..+4."""
                half, ic, j0 = step // 16, (step // 4) % 4, 4 * (step % 4)
                po = (h % 2) * D
                yps, pt = state["yps"], pt_sb[h]
                for jt in range(j0, j0 + 4):
                    nc.tensor.matmul(
                        yps[:, ic, 0:D + 1],
                        pt[:, jt, (half * 4 + ic) * 128:(half * 4 + ic + 1) * 128],
                        vt_sb[jt][:, h, :],
                        start=(jt == 0 and ic == 0),
                        stop=(jt == NJ - 1),
                        skip_group_check=True)
                if step % 16 == 15:
                    if half == 0:
                        inv = invp.tile([128, 8], F32, tag="inv",
                                        name=f"inv{h}")
                        state["inv"] = inv
                    else:
                        inv = state["inv"]
                    nc.vector.reciprocal(
                        inv[:, half * 4:(half + 1) * 4],
                        yps[:, :, D:D + 1].rearrange("p a b -> p (a b)"))
                    nc.vector.tensor_mul(
                        yts_sb[h // 2][:, half * 4:(half + 1) * 4, po:po + D],
                        yps[:, :, 0:D],
                        inv[:, half * 4:(half + 1) * 4].unsqueeze(2)
                        .broadcast_to((128, 4, D)))
                    if half == 0:
                        state["yps"] = ps_yt.tile([128, 4, 128], F32,
                                                  tag="yt", name=f"yt{h}_1")

            def transpose_pair(pair):
                rt = pstile(f"y4ps{pair}")
                y4ps = rt[:].bitcast(BF16)[:, 0:LQ]
                for ich in range(NI):
                    nc.tensor.transpose(y4ps[:, ich * 128:(ich + 1) * 128],
                                        yts_sb[pair][:, ich, :], idn_sb[:])
                y4 = y4p.tile([128, LQ], BF16, tag="y4", name=f"y4_{pair}")
                nc.vector.tensor_copy(y4[:], y4ps)
                y4_sb.append(y4)

            for h in range(H + 1):
                if h < H:
                    pt_sb[h] = ptp.tile([128, NJ, LQ], BF16, tag="pt",
                                        name=f"pt{h}")
                    if h % 2 == 0:
                        yts_sb[h // 2] = ytsp.tile([128, NI, 128], BF16,
                                                   tag="yts",
                                                   name=f"yts{h // 2}")
                if h > 0:
                    yt_start(h - 1)
                for jt in range(NJ):
                    if h < H:
                        sim_chunk(h, jt)
                    if h == 0 and jt >= 8:
                        make_vt(jt)
                    elif h in (1, 2, 3):
                        mt = h
                        if jt == 2:
                            make_q(mt)
                        elif jt == 6:
                            make_k_half(mt, 0,
                                        nc.scalar if mt % 2 == 1 else nc.vector)
                        elif jt == 10:
                            make_k_half(mt, 1,
                                        nc.vector if mt % 2 == 1 else nc.scalar)
                    if h > 0 and jt > 0:
                        yt_step(h - 1, 2 * jt - 2)
                        yt_step(h - 1, 2 * jt - 1)
                        if jt == NJ - 1:
                            yt_step(h - 1, 30)
                            yt_step(h - 1, 31)
                    if jt == 6 and h > 1 and h % 2 == 1:
                        transpose_pair(h // 2 - 1)
                    if h == H and jt in (2, 4):
                        mt = 0 if jt == 2 else 1
                        ops = pstile(f"ops{mt}")
                        out_ps.append(ops)
                        for pair in range(3):
                            for n in range(LQ // 512):
                                nc.tensor.matmul(
                                    ops[:, n * 512:(n + 1) * 512],
                                    wo_sb[pair][:, mt * 128:(mt + 1) * 128],
                                    y4_sb[pair][:, n * 512:(n + 1) * 512],
                                    start=(pair == 0), stop=False)
            transpose_pair(3)

            # ------- out-proj tail + bias + final rms-norm (in-ring) -------
            sq2 = sqp.tile([128, 2, LQ], BF16, tag="sq2")
            ss2r = pstile("ss2ps")
            out_sb = []
            for mt in range(2):
                for n in range(LQ // 512):
                    nc.tensor.matmul(
                        out_ps[mt][:, n * 512:(n + 1) * 512],
                        wo_sb[3][:, mt * 128:(mt + 1) * 128],
                        y4_sb[3][:, n * 512:(n + 1) * 512],
                        start=False, stop=True)
                t = outp.tile([128, LQ], BF16, tag="osb")
                nc.vector.tensor_scalar_add(t[:], out_ps[mt][:, :],
                                            bo_sb[mt][:])
                out_sb.append(t)
                nc.vector.tensor_mul(sq2[:, mt, :], t[:], t[:])
                for n in range(LQ // 512):
                    nc.tensor.matmul(ss2r[0:1, n * 512:(n + 1) * 512],
                                     ones_col[:],
                                     sq2[:, mt, n * 512:(n + 1) * 512],
                                     start=(mt == 0), stop=(mt == 1))
            s2tmp = widep.tile([1, LQ], F32, tag="stmp", name="s2tmp")
            nc.scalar.activation(s2tmp[:], ss2r[0:1, :], AF.Sqrt,
                                 bias=eps_t[:], scale=1.0 / C)
            s2 = widep.tile([1, LQ], F32R, tag="wide", name="s2")
            nc.vector.reciprocal(s2[:], s2tmp[:])
            bc2r = pstile("bc2ps")
            for n in range(LQ // 512):
                nc.tensor.matmul(bc2r[:, n * 512:(n + 1) * 512],
                                 ones_row[:], s2[0:1, n * 512:(n + 1) * 512],
                                 start=True, stop=True)
            for mt in range(2):
                t = finp.tile([128, LQ], F32, tag="fin")
                for half in range(2):
                    hs = slice(half * 512, (half + 1) * 512)
                    nc.vector.scalar_tensor_tensor(
                        t[:, hs], out_sb[mt][:, hs], g2_sb[mt][:],
                        bc2r[:, hs], op0=MUL, op1=MUL)
                    nc.sync.dma_start(out[mt * 128:(mt + 1) * 128, hs],
                                      t[:, hs])




_NC = None


def _get_nc():
    global _NC
    if _NC is None:
        nc = bacc.Bacc("TRN2", target_bir_lowering=False, debug=False,
                       enable_asserts=False, num_devices=8)
        x_d = nc.dram_tensor("x", [C, L], BF16, kind="ExternalInput")
        wq_d = nc.dram_tensor("wqkvT", [C, 3 * HID], BF16, kind="ExternalInput")
        wo_d = nc.dram_tensor("woutT", [HID, C], BF16, kind="ExternalInput")
        b_d = nc.dram_tensor("bout", [C, 1], F32, kind="ExternalInput")
        g2_d = nc.dram_tensor("g2v", [C, 1], F32, kind="ExternalInput")
        idn_d = nc.dram_tensor("idn", [128, 128], BF16, kind="ExternalInput")
        out_d = nc.dram_tensor("out", [C, LQ], F32, kind="ExternalOutput")
        with tile.TileContext(nc) as tc:
            _body(tc, x_d.ap(), wq_d.ap(), wo_d.ap(), b_d.ap(), g2_d.ap(),
                  idn_d.ap(), out_d.ap())
        nc.compile()
        _NC = nc
    return _NC


def _in_maps(x, g1, w_qkv, w_out, b_out, g2):
    BFH = ml_dtypes.bfloat16
    w2 = (np.asarray(w_qkv, np.float32)
          * np.asarray(g1, np.float32).reshape(1, C))
    wqkvT = np.ascontiguousarray(w2.T).astype(BFH)
    woutT = np.ascontiguousarray(np.asarray(w_out, np.float32).T).astype(BFH)
    bo = np.asarray(b_out, np.float32).reshape(C, 1)
    g2v = np.asarray(g2, np.float32).reshape(C, 1)
    idn = np.eye(128, dtype=BFH)
    maps = []
    for core in range(8):
        b, half = divmod(core, 2)
        xb = np.asarray(x[b], np.float32)
        x_core = np.ascontiguousarray(np.concatenate(
            [xb[:, half * LQ:(half + 1) * LQ],
             xb[:, (1 - half) * LQ:(2 - half) * LQ]], axis=1)).astype(BFH)
        maps.append({"x": x_core, "wqkvT": wqkvT, "woutT": woutT,
                     "bout": bo, "g2v": g2v, "idn": idn})
    return maps


def _assemble(results):
    out = np.empty((B, C, L), np.float32)
    for core in range(8):
        b, half = divmod(core, 2)
        out[b][:, half * LQ:(half + 1) * LQ] = results[core]["out"]
    return out


def kernel(x, g1, w_qkv, w_out, b_out, g2, _trace=False, _tmpdir=None):
    res = run_bass_kernel_spmd(_get_nc(),
                               _in_maps(x, g1, w_qkv, w_out, b_out, g2),
                               core_ids=list(range(8)), trace=_trace,
                               tmpdir=_tmpdir)
    out = _assemble(res.results)
    if _trace:
        return out, res
    return out


# revision 72
# speedup vs baseline: 1.0015x; 1.0015x over previous
"""Trainium2 Bass kernel for the attention module (b=4, c=256, l=2048, h=8, d=64).

Sharding: 8 cores = 4 batches x 2 query-halves (no collectives). Each core
receives its batch's x with columns permuted so its own query half comes
first; it computes k/v for all 2048 key positions and the attention output
for its 1024 queries, then the output projection + final rms-norm.

Device algorithm per core (all matmul data bf16, psum fp32):
  ss_j   = sum_c bf16(x[c,j]^2)              (PE ones-reduce)
  s_j    = 1/sqrt(ss_j/256 + eps)            (ACT sqrt + DVE reciprocal)
  xn     = bf16(x * bcast(s))                (PE ones-broadcast + DVE)
  q,k    = bf16(wqkvT-slices.T @ xn)         (g1 folded on host)
  vT     = bf16(xn_slice.T @ wv), ones col fused at index 64
  per head:  simT[j,i] = k_h.T @ q_h         (psum fp32, logits*8)
             pt = exp(simT/8)  as bf16       (split ~9/7 between ACT true
                                              exp and DVE Schraudolph
                                              bitcast-exp via uint16)
             ytps[i, d|den] += pt_chunk.T @ [vT_h|1]   (transposed y, fused den)
  y_h    = ytps[:, 0:64] * (1/den) broadcast (DVE, stride-0 bcast)
  y4     = PE transpose(yts, identity)       (bf16, per head-pair)
  out    = woT.T @ y4 + b_out
  result = out * bcast(1/sqrt(ss2/256+eps)) * g2

Scheduling: one rotating 3x[128,1024] psum arena serves the norm, all
projections, sim chunks, the pair transposes, and the out-proj/final-norm
tail, so no phase blocks on a psum-bank WAR against another pool. Head h's
sim+exp stream overlaps head h-1's yT accumulation (whose pt is fully
materialized, so yT never waits on an exp producer). exp jts interleave
across ACT/DVE so both engines drain the triple-buffered ring in parallel.
Late-consumed tensors (idn/wo/bo/g2) are DMA'd on the gpsimd SWDGE queue
because a queue's waiters effectively wait on the whole queue's clock.
"""
import sys

import numpy as np

if "/opt/trn_rl_repo" not in sys.path:
    sys.path.insert(0, "/opt/trn_rl_repo")

import ml_dtypes  # noqa: E402

import concourse.bass as bass  # noqa: E402
import concourse.tile as tile  # noqa: E402
from concourse import bacc, mybir  # noqa: E402
from concourse.bass_utils import run_bass_kernel_spmd  # noqa: E402

F32 = mybir.dt.float32
F32R = mybir.dt.float32r
BF16 = mybir.dt.bfloat16
FP8 = mybir.dt.float8e4
U16 = mybir.dt.uint16
AF = mybir.ActivationFunctionType
DR = mybir.MatmulPerfMode.DoubleRow
MUL = mybir.AluOpType.mult
ADD = mybir.AluOpType.add

B, C, L = 4, 256, 2048
H, D = 8, 64
HID = H * D
LQ = L // 2      # queries per core
NCT = C // 128   # 2 c-tiles
NJ = L // 128    # 16 j-tiles
NI = LQ // 128   # 8 i-chunks
EPS_B = 1e-26

LOG2E = 1.4426950408889634
# Schraudolph bf16: bits = round(logit*128*log2e + 127*128 - 5.6); logit = sim/8
SCH_A = 128.0 * LOG2E / 8.0
SCH_B = 127.0 * 128.0 - 5.6

# exp engine split: jts are interleaved across ACT (true exp) and DVE
# (Schraudolph bitcast-exp) so both engines drain the sim psum in parallel;
# any 3 consecutive jts touch both engines (sim psum is triple-buffered).
# gpsimd cannot read PSUM on TRN2, so it gets no exp work.
DVE_JT = frozenset((1, 3, 5, 7, 9, 12, 14))


def _body(tc, x, wq, wo, bo, g2, idn, out):
    nc = tc.nc
    from contextlib import ExitStack
    with ExitStack() as ctx:
        ctx.enter_context(nc.allow_low_precision(
            reason="bf16/fp8 data path by design"))
        const = ctx.enter_context(tc.tile_pool(name="const", bufs=1))
        big = ctx.enter_context(tc.tile_pool(name="big", bufs=2))
        xnp = ctx.enter_context(tc.tile_pool(name="xn", bufs=4))
        sqp = ctx.enter_context(tc.tile_pool(name="sq", bufs=1))
        qp = ctx.enter_context(tc.tile_pool(name="q", bufs=4))
        kp = ctx.enter_context(tc.tile_pool(name="k", bufs=4))
        vtp = ctx.enter_context(tc.tile_pool(name="vt", bufs=16))
        ptp = ctx.enter_context(tc.tile_pool(name="pt", bufs=2))
        invp = ctx.enter_context(tc.tile_pool(name="inv", bufs=2))
        ytsp = ctx.enter_context(tc.tile_pool(name="yts", bufs=2))
        y4p = ctx.enter_context(tc.tile_pool(name="y4", bufs=4))
        widep = ctx.enter_context(tc.tile_pool(name="wide", bufs=2))
        outp = ctx.enter_context(tc.tile_pool(name="outp", bufs=2))
        finp = ctx.enter_context(tc.tile_pool(name="fin", bufs=2))

        # ---------------- constants & weights ----------------
        stage = const.tile([128, 8], F32, tag="stage")
        nc.vector.memset(stage[:], 1.0)
        ones_col = const.tile([128, 1], BF16, tag="ones_col")
        nc.vector.tensor_copy(ones_col[:], stage[:, 0:1])
        ones_row = const.tile([1, 128], F32R, tag="ones_row")
        nc.vector.tensor_copy(ones_row[:],
                              stage[0:1, 0:1].broadcast_to((1, 128)))
        eps_t = const.tile([1, 1], F32, tag="eps")
        nc.vector.memset(eps_t[:], EPS_B)
        zero_t = const.tile([128, 1], F32, tag="zero")
        nc.vector.memset(zero_t[:], 0.0)

        # Critical-path DMAs (x, wq) ride the ACT HWDGE queue alone: waiters
        # of a queue's completion sem effectively wait for the whole queue,
        # so late-consumed tensors go on SP instead.
        x_sb = []
        for ct in range(NCT):
            t = big.tile([128, L], BF16, tag="big")
            x_sb.append(t)
        for half in range(2):
            for ct in range(NCT):
                nc.sync.dma_start(
                    x_sb[ct][:, half * 1024:(half + 1) * 1024],
                    x[ct * 128:(ct + 1) * 128,
                      half * 1024:(half + 1) * 1024])

        wq_sb = []
        for ct in range(NCT):
            t = const.tile([128, 3 * HID], BF16, tag=f"wq{ct}")
            nc.sync.dma_start(t[:], wq[ct * 128:(ct + 1) * 128, :])
            wq_sb.append(t)
        # idn/wo/bo/g2 are needed only late; their dma_starts are emitted
        # after the projections so early consumers don't wait on them (DMA
        # completion sems are threshold-counted per queue).
        idn_sb = const.tile([128, 128], BF16, tag="idn")
        wo_sb = [const.tile([128, C], BF16, tag=f"wo{kt}", name=f"wo{kt}")
                 for kt in range(4)]
        bo_sb = [const.tile([128, 1], F32, tag=f"bo{mt}", name=f"bo{mt}")
                 for mt in range(2)]
        g2_sb = [const.tile([128, 1], F32, tag=f"g2{mt}", name=f"g2{mt}")
                 for mt in range(2)]

        def _late_dmas():
            # Pool SWDGE: keeps the SP HWDGE queue (x, wq, out) clean so
            # early consumers' coalesced DMA-clock waits release early.
            nc.gpsimd.dma_start(idn_sb[:], idn)
            for kt in range(4):
                nc.gpsimd.dma_start(wo_sb[kt][:],
                                    wo[kt * 128:(kt + 1) * 128, :])
            for mt in range(2):
                nc.gpsimd.dma_start(bo_sb[mt][:],
                                    bo[mt * 128:(mt + 1) * 128, :])
                nc.gpsimd.dma_start(g2_sb[mt][:],
                                    g2[mt * 128:(mt + 1) * 128, :])

        # ------------- unified psum pipeline: norm + proj + attention ------
        # ONE rotating psum arena (3 x [128,1024]) serves the norm's ss/bc,
        # all projections, and the sim chunks, so no phase ever blocks on a
        # psum-bank WAR against a different pool's slowest reader.
        xn_sb = [[None] * 2 for _ in range(NCT)]
        q_sb, k_sb = [None] * 4, [None] * 4
        vt_sb = [None] * NJ
        y4_sb = []
        with tc.tile_pool(name="ps", bufs=3, space="PSUM") as psp, \
                tc.tile_pool(name="ps_yt", bufs=2, space="PSUM") as ps_yt:

            def pstile(name):
                return psp.tile([128, 1024], F32, tag="sim", name=name)

            # --- input rms-norm: per-engine stage batching so the DVE
            # queue never buries a later half behind an earlier half ---
            s1 = widep.tile([1, L], F32R, tag="wide")
            sq8t, sst, stmpt, bct = [], [], [], []
            for half in range(2):
                hs = slice(half * 1024, (half + 1) * 1024)
                t = sqp.tile([128, 2, 1024], BF16, tag="sq8",
                             name=f"sq8_{half}")
                for ct in range(NCT):
                    nc.vector.tensor_mul(t[:, ct, :],
                                         x_sb[ct][:, hs], x_sb[ct][:, hs])
                sq8t.append(t)
            for half in range(2):
                sst.append(pstile(f"ssps{half}"))
                for n in range(2):
                    for ct in range(NCT):
                        nc.tensor.matmul(
                            sst[half][0:1, n * 512:(n + 1) * 512],
                            ones_col[:],
                            sq8t[half][:, ct, n * 512:(n + 1) * 512],
                            start=(ct == 0), stop=(ct == NCT - 1))
            for half in range(2):
                stmp = widep.tile([1, 1024], F32, tag="stmp",
                                  name=f"stmp{half}")
                nc.scalar.activation(stmp[:], sst[half][0:1, :], AF.Sqrt,
                                     bias=eps_t[:], scale=1.0 / C)
                stmpt.append(stmp)
            for half in range(2):
                hs = slice(half * 1024, (half + 1) * 1024)
                nc.vector.reciprocal(s1[0:1, hs], stmpt[half][:])
            for half in range(2):
                bct.append(pstile(f"bcps{half}"))
                for n in range(2):
                    cs = slice(half * 1024 + n * 512,
                               half * 1024 + (n + 1) * 512)
                    nc.tensor.matmul(bct[half][:, n * 512:(n + 1) * 512],
                                     ones_row[:], s1[0:1, cs],
                                     start=True, stop=True)
            for half in range(2):
                hs = slice(half * 1024, (half + 1) * 1024)
                for ct in range(NCT):
                    t = xnp.tile([128, 1024], BF16, tag="xn",
                                 name=f"xn{ct}_{half}")
                    nc.vector.tensor_mul(t[:], x_sb[ct][:, hs],
                                         bct[half][:, :])
                    xn_sb[ct][half] = t

            # --- projection helpers (psum from the shared arena) ---
            def make_q(mt):
                ps = pstile(f"qps{mt}")
                for n in range(LQ // 512):
                    for ct in range(NCT):
                        nc.tensor.matmul(
                            ps[:, n * 512:(n + 1) * 512],
                            wq_sb[ct][:, mt * 128:(mt + 1) * 128],
                            xn_sb[ct][0][:, n * 512:(n + 1) * 512],
                            start=(ct == 0), stop=(ct == NCT - 1))
                t = qp.tile([128, LQ], BF16, tag="q", name=f"qsb{mt}")
                nc.scalar.copy(t[:], ps[:, :])
                q_sb[mt] = t

            def make_k_half(mt, half, eng):
                if half == 0:
                    k_sb[mt] = kp.tile([128, L], BF16, tag="k",
                                       name=f"ksb{mt}")
                t = k_sb[mt]
                ps = pstile(f"kps{mt}_{half}")
                for n in range(2):
                    for ct in range(NCT):
                        nc.tensor.matmul(
                            ps[:, n * 512:(n + 1) * 512],
                            wq_sb[ct][:, HID + mt * 128:HID + (mt + 1) * 128],
                            xn_sb[ct][half][:, n * 512:(n + 1) * 512],
                            start=(ct == 0), stop=(ct == NCT - 1))
                if eng is nc.scalar:
                    eng.copy(t[:, half * LQ:(half + 1) * LQ], ps[:, :])
                else:
                    eng.tensor_copy(t[:, half * LQ:(half + 1) * LQ], ps[:, :])

            def make_vt(jt):
                ps = pstile(f"vps{jt}")
                for ct in range(NCT):
                    nc.tensor.matmul(ps[:, 0:512],
                                     xn_sb[ct][jt // 8][:, (jt % 8) * 128:
                                                        (jt % 8 + 1) * 128],
                                     wq_sb[ct][:, 2 * HID:3 * HID],
                                     start=(ct == 0), stop=(ct == NCT - 1))
                t = vtp.tile([128, H, D + 1], BF16, tag="vt", name=f"vt{jt}")
                src = ps[:, 0:512].rearrange("p (h e) -> p h e", e=D)
                if jt % 2 == 0:
                    nc.vector.tensor_copy(t[:, :, 0:D], src)
                else:
                    nc.scalar.copy(t[:, :, 0:D], src)
                nc.gpsimd.tensor_copy(
                    t[:, :, D:D + 1].rearrange("p h o -> p (h o)"),
                    stage[:, 0:H])
                vt_sb[jt] = t

            make_q(0)
            make_k_half(0, 0, nc.vector)
            make_k_half(0, 1, nc.scalar)
            _late_dmas()
            for jt in range(8):
                make_vt(jt)

            # --- attention: head h sims + exp stream while head h-1's yT
            # accumulates from its fully-materialized pt tile; projections
            # for later heads interleave into h0-h3's slots. ---
            pt_sb = [None] * H
            yts_sb = [None] * 4
            out_ps = []
            state = {}

            def sim_chunk(h, jt):
                mt, po = h // 2, (h % 2) * D
                sps = pstile(f"sps{h}_{jt}")
                for n in range(LQ // 512):
                    nc.tensor.matmul(
                        sps[:, n * 512:(n + 1) * 512],
                        k_sb[mt][po:po + D, jt * 128:(jt + 1) * 128],
                        q_sb[mt][po:po + D, n * 512:(n + 1) * 512],
                        start=True, stop=True)
                dst = pt_sb[h][:, jt, :]
                if jt in DVE_JT:
                    nc.vector.tensor_scalar(
                        dst.bitcast(U16), sps[:, :], SCH_A, SCH_B,
                        op0=MUL, op1=ADD)
                else:
                    nc.scalar.activation(dst, sps[:, :], AF.Exp,
                                         bias=zero_t[:], scale=0.125)

            def yt_start(h):
                state["yps"] = ps_yt.tile([128, 4, 128], F32, tag="yt",
                                          name=f"yt{h}_0")

            def yt_step(h, step):
                """steps 0..31: (half, ic, jts 0..15) in 8-instr groups.
                step s covers half=s//16, ic=(s//4)%4, jts 4*(s%4)..+4."""
                half, ic, j0 = step // 16, (step // 4) % 4, 4 * (step % 4)
                po = (h % 2) * D
                yps, pt = state["yps"], pt_sb[h]
                for jt in range(j0, j0 + 4):
                    nc.tensor.matmul(
                        yps[:, ic, 0:D + 1],
                        pt[:, jt, (half * 4 + ic) * 128:(half * 4 + ic + 1) * 128],
                        vt_sb[jt][:, h, :],
                        start=(jt == 0 and ic == 0),
                        stop=(jt == NJ - 1),
                        skip_group_check=True)
                if step % 16 == 15:
                    if half == 0:
                        inv = invp.tile([128, 8], F32, tag="inv",
                                        name=f"inv{h}")
                        state["inv"] = inv
                    else:
                        inv = state["inv"]
                    nc.vector.reciprocal(
                        inv[:, half * 4:(half + 1) * 4],
                        yps[:, :, D:D + 1].rearrange("p a b -> p (a b)"))
                    nc.vector.tensor_mul(
                        yts_sb[h // 2][:, half * 4:(half + 1) * 4, po:po + D],
                        yps[:, :, 0:D],
                        inv[:, half * 4:(half + 1) * 4].unsqueeze(2)
                        .broadcast_to((128, 4, D)))
                    if half == 0:
                        state["yps"] = ps_yt.tile([128, 4, 128], F32,
                                                  tag="yt", name=f"yt{h}_1")

            def transpose_pair(pair):
                rt = pstile(f"y4ps{pair}")
                y4ps = rt[:].bitcast(BF16)[:, 0:LQ]
                for ich in range(NI):
                    nc.tensor.transpose(y4ps[:, ich * 128:(ich + 1) * 128],
                                        yts_sb[pair][:, ich, :], idn_sb[:])
                y4 = y4p.tile([128, LQ], BF16, tag="y4", name=f"y4_{pair}")
                nc.vector.tensor_copy(y4[:], y4ps)
                y4_sb.append(y4)

            for h in range(H + 1):
                if h < H:
                    pt_sb[h] = ptp.tile([128, NJ, LQ], BF16, tag="pt",
                                        name=f"pt{h}")
                    if h % 2 == 0:
                        yts_sb[h // 2] = ytsp.tile([128, NI, 128], BF16,
                                                   tag="yts",
                                                   name=f"yts{h // 2}")
                if h > 0:
                    yt_start(h - 1)
                for jt in range(NJ):
                    if h < H:
                        sim_chunk(h, jt)
                    if h == 0 and jt >= 8:
                        make_vt(jt)
                    elif h in (1, 2, 3):
                        mt = h
                        if jt == 2:
                            make_q(mt)
                        elif jt == 6:
                            make_k_half(mt, 0,
                                        nc.scalar if mt % 2 == 1 else nc.vector)
                        elif jt == 10:
                            make_k_half(mt, 1,
                                        nc.vector if mt % 2 == 1 else nc.scalar)
                    if h > 0 and jt > 0:
                        yt_step(h - 1, 2 * jt - 2)
                        yt_step(h - 1, 2 * jt - 1)
                        if jt == NJ - 1:
                            yt_step(h - 1, 30)
                            yt_step(h - 1, 31)
                    if jt == 4 and h > 1 and h % 2 == 1:
                        transpose_pair(h // 2 - 1)
                    if h == H and jt in (2, 4):
                        mt = 0 if jt == 2 else 1
                        ops = pstile(f"ops{mt}")
                        out_ps.append(ops)
                        for pair in range(3):
                            for n in range(LQ // 512):
                                nc.tensor.matmul(
                                    ops[:, n * 512:(n + 1) * 512],
                                    wo_sb[pair][:, mt * 128:(mt + 1) * 128],
                                    y4_sb[pair][:, n * 512:(n + 1) * 512],
                                    start=(pair == 0), stop=False)
            transpose_pair(3)

            # ------- out-proj tail + bias + final rms-norm (in-ring) -------
            sq2 = sqp.tile([128, 2, LQ], BF16, tag="sq2")
            ss2r = pstile("ss2ps")
            out_sb = []
            for mt in range(2):
                for n in range(LQ // 512):
                    nc.tensor.matmul(
                        out_ps[mt][:, n * 512:(n + 1) * 512],
                        wo_sb[3][:, mt * 128:(mt + 1) * 128],
                        y4_sb[3][:, n * 512:(n + 1) * 512],
                        start=False, stop=True)
                t = outp.tile([128, LQ], BF16, tag="osb")
                nc.vector.tensor_scalar_add(t[:], out_ps[mt][:, :],
                                            bo_sb[mt][:])
                out_sb.append(t)
                nc.vector.tensor_mul(sq2[:, mt, :], t[:], t[:])
                for n in range(LQ // 512):
                    nc.tensor.matmul(ss2r[0:1, n * 512:(n + 1) * 512],
                                     ones_col[:],
                                     sq2[:, mt, n * 512:(n + 1) * 512],
                                     start=(mt == 0), stop=(mt == 1))
            s2tmp = widep.tile([1, LQ], F32, tag="stmp", name="s2tmp")
            nc.scalar.activation(s2tmp[:], ss2r[0:1, :], AF.Sqrt,
                                 bias=eps_t[:], scale=1.0 / C)
            s2 = widep.tile([1, LQ], F32R, tag="wide", name="s2")
            nc.vector.reciprocal(s2[:], s2tmp[:])
            bc2r = pstile("bc2ps")
            for n in range(LQ // 512):
                nc.tensor.matmul(bc2r[:, n * 512:(n + 1) * 512],
                                 ones_row[:], s2[0:1, n * 512:(n + 1) * 512],
                                 start=True, stop=True)
            for mt in range(2):
                t = finp.tile([128, LQ], F32, tag="fin")
                for half in range(2):
                    hs = slice(half * 512, (half + 1) * 512)
                    nc.vector.scalar_tensor_tensor(
                        t[:, hs], out_sb[mt][:, hs], g2_sb[mt][:],
                        bc2r[:, hs], op0=MUL, op1=MUL)
                    nc.sync.dma_start(out[mt * 128:(mt + 1) * 128, hs],
                                      t[:, hs])




_NC = None


def _get_nc():
    global _NC
    if _NC is None:
        nc = bacc.Bacc("TRN2", target_bir_lowering=False, debug=False,
                       enable_asserts=False, num_devices=8)
        x_d = nc.dram_tensor("x", [C, L], BF16, kind="ExternalInput")
        wq_d = nc.dram_tensor("wqkvT", [C, 3 * HID], BF16, kind="ExternalInput")
        wo_d = nc.dram_tensor("woutT", [HID, C], BF16, kind="ExternalInput")
        b_d = nc.dram_tensor("bout", [C, 1], F32, kind="ExternalInput")
        g2_d = nc.dram_tensor("g2v", [C, 1], F32, kind="ExternalInput")
        idn_d = nc.dram_tensor("idn", [128, 128], BF16, kind="ExternalInput")
        out_d = nc.dram_tensor("out", [C, LQ], F32, kind="ExternalOutput")
        with tile.TileContext(nc) as tc:
            _body(tc, x_d.ap(), wq_d.ap(), wo_d.ap(), b_d.ap(), g2_d.ap(),
                  idn_d.ap(), out_d.ap())
        nc.compile()
        _NC = nc
    return _NC


def _in_maps(x, g1, w_qkv, w_out, b_out, g2):
    BFH = ml_dtypes.bfloat16
    w2 = (np.asarray(w_qkv, np.float32)
          * np.asarray(g1, np.float32).reshape(1, C))
    wqkvT = np.ascontiguousarray(w2.T).astype(BFH)
    woutT = np.ascontiguousarray(np.asarray(w_out, np.float32).T).astype(BFH)
    bo = np.asarray(b_out, np.float32).reshape(C, 1)
    g2v = np.asarray(g2, np.float32).reshape(C, 1)
    idn = np.eye(128, dtype=BFH)
    maps = []
    for core in range(8):
        b, half = divmod(core, 2)
        xb = np.asarray(x[b], np.float32)
        x_core = np.ascontiguousarray(np.concatenate(
            [xb[:, half * LQ:(half + 1) * LQ],
             xb[:, (1 - half) * LQ:(2 - half) * LQ]], axis=1)).astype(BFH)
        maps.append({"x": x_core, "wqkvT": wqkvT, "woutT": woutT,
                     "bout": bo, "g2v": g2v, "idn": idn})
    return maps


def _assemble(results):
    out = np.empty((B, C, L), np.float32)
    for core in range(8):
        b, half = divmod(core, 2)
        out[b][:, half * LQ:(half + 1) * LQ] = results[core]["out"]
    return out


def kernel(x, g1, w_qkv, w_out, b_out, g2, _trace=False, _tmpdir=None):
    res = run_bass_kernel_spmd(_get_nc(),
                               _in_maps(x, g1, w_qkv, w_out, b_out, g2),
                               core_ids=list(range(8)), trace=_trace,
                               tmpdir=_tmpdir)
    out = _assemble(res.results)
    if _trace:
        return out, res
    return out


# revision 73
# speedup vs baseline: 1.0040x; 1.0025x over previous
"""Trainium2 Bass kernel for the attention module (b=4, c=256, l=2048, h=8, d=64).

Sharding: 8 cores = 4 batches x 2 query-halves (no collectives). Each core
receives its batch's x with columns permuted so its own query half comes
first; it computes k/v for all 2048 key positions and the attention output
for its 1024 queries, then the output projection + final rms-norm.

Device algorithm per core (all matmul data bf16, psum fp32):
  ss_j   = sum_c bf16(x[c,j]^2)              (PE ones-reduce)
  s_j    = 1/sqrt(ss_j/256 + eps)            (ACT sqrt + DVE reciprocal)
  xn     = bf16(x * bcast(s))                (PE ones-broadcast + DVE)
  q,k    = bf16(wqkvT-slices.T @ xn)         (g1 folded on host)
  vT     = bf16(xn_slice.T @ wv), ones col fused at index 64
  per head:  simT[j,i] = k_h.T @ q_h         (psum fp32, logits*8)
             pt = exp(simT/8)  as bf16       (split ~9/7 between ACT true
                                              exp and DVE Schraudolph
                                              bitcast-exp via uint16)
             ytps[i, d|den] += pt_chunk.T @ [vT_h|1]   (transposed y, fused den)
  y_h    = ytps[:, 0:64] * (1/den) broadcast (DVE, stride-0 bcast)
  y4     = PE transpose(yts, identity)       (bf16, per head-pair)
  out    = woT.T @ y4 + b_out
  result = out * bcast(1/sqrt(ss2/256+eps)) * g2

Scheduling: one rotating 3x[128,1024] psum arena serves the norm, all
projections, sim chunks, the pair transposes, and the out-proj/final-norm
tail, so no phase blocks on a psum-bank WAR against another pool. Head h's
sim+exp stream overlaps head h-1's yT accumulation (whose pt is fully
materialized, so yT never waits on an exp producer). exp jts interleave
across ACT/DVE so both engines drain the triple-buffered ring in parallel.
Late-consumed tensors (idn/wo/bo/g2) are DMA'd on the gpsimd SWDGE queue
because a queue's waiters effectively wait on the whole queue's clock.
"""
import sys

import numpy as np

if "/opt/trn_rl_repo" not in sys.path:
    sys.path.insert(0, "/opt/trn_rl_repo")

import ml_dtypes  # noqa: E402

import concourse.bass as bass  # noqa: E402
import concourse.tile as tile  # noqa: E402
from concourse import bacc, mybir  # noqa: E402
from concourse.bass_utils import run_bass_kernel_spmd  # noqa: E402

F32 = mybir.dt.float32
F32R = mybir.dt.float32r
BF16 = mybir.dt.bfloat16
FP8 = mybir.dt.float8e4
U16 = mybir.dt.uint16
AF = mybir.ActivationFunctionType
DR = mybir.MatmulPerfMode.DoubleRow
MUL = mybir.AluOpType.mult
ADD = mybir.AluOpType.add

B, C, L = 4, 256, 2048
H, D = 8, 64
HID = H * D
LQ = L // 2      # queries per core
NCT = C // 128   # 2 c-tiles
NJ = L // 128    # 16 j-tiles
NI = LQ // 128   # 8 i-chunks
EPS_B = 1e-26

LOG2E = 1.4426950408889634
# Schraudolph bf16: bits = round(logit*128*log2e + 127*128 - 5.6); logit = sim/8
SCH_A = 128.0 * LOG2E / 8.0
SCH_B = 127.0 * 128.0 - 5.6

# exp engine split: jts are interleaved across ACT (true exp) and DVE
# (Schraudolph bitcast-exp) so both engines drain the sim psum in parallel;
# any 3 consecutive jts touch both engines (sim psum is triple-buffered).
# gpsimd cannot read PSUM on TRN2, so it gets no exp work.
DVE_JT = frozenset((1, 3, 5, 7, 9, 12, 14))


def _body(tc, x, wq, wo, bo, g2, idn, out):
    nc = tc.nc
    from contextlib import ExitStack
    with ExitStack() as ctx:
        ctx.enter_context(nc.allow_low_precision(
            reason="bf16/fp8 data path by design"))
        const = ctx.enter_context(tc.tile_pool(name="const", bufs=1))
        big = ctx.enter_context(tc.tile_pool(name="big", bufs=2))
        xnp = ctx.enter_context(tc.tile_pool(name="xn", bufs=4))
        sqp = ctx.enter_context(tc.tile_pool(name="sq", bufs=2))
        qp = ctx.enter_context(tc.tile_pool(name="q", bufs=4))
        kp = ctx.enter_context(tc.tile_pool(name="k", bufs=4))
        vtp = ctx.enter_context(tc.tile_pool(name="vt", bufs=16))
        ptp = ctx.enter_context(tc.tile_pool(name="pt", bufs=2))
        invp = ctx.enter_context(tc.tile_pool(name="inv", bufs=2))
        ytsp = ctx.enter_context(tc.tile_pool(name="yts", bufs=2))
        y4p = ctx.enter_context(tc.tile_pool(name="y4", bufs=4))
        widep = ctx.enter_context(tc.tile_pool(name="wide", bufs=2))
        outp = ctx.enter_context(tc.tile_pool(name="outp", bufs=2))
        finp = ctx.enter_context(tc.tile_pool(name="fin", bufs=2))

        # ---------------- constants & weights ----------------
        stage = const.tile([128, 8], F32, tag="stage")
        nc.vector.memset(stage[:], 1.0)
        ones_col = const.tile([128, 1], BF16, tag="ones_col")
        nc.vector.tensor_copy(ones_col[:], stage[:, 0:1])
        ones_row = const.tile([1, 128], F32R, tag="ones_row")
        nc.vector.tensor_copy(ones_row[:],
                              stage[0:1, 0:1].broadcast_to((1, 128)))
        eps_t = const.tile([1, 1], F32, tag="eps")
        nc.vector.memset(eps_t[:], EPS_B)
        zero_t = const.tile([128, 1], F32, tag="zero")
        nc.vector.memset(zero_t[:], 0.0)

        # Critical-path DMAs (x, wq) ride the ACT HWDGE queue alone: waiters
        # of a queue's completion sem effectively wait for the whole queue,
        # so late-consumed tensors go on SP instead.
        x_sb = []
        for ct in range(NCT):
            t = big.tile([128, L], BF16, tag="big")
            x_sb.append(t)
        for half in range(2):
            for ct in range(NCT):
                nc.sync.dma_start(
                    x_sb[ct][:, half * 1024:(half + 1) * 1024],
                    x[ct * 128:(ct + 1) * 128,
                      half * 1024:(half + 1) * 1024])

        wq_sb = []
        for ct in range(NCT):
            t = const.tile([128, 3 * HID], BF16, tag=f"wq{ct}")
            nc.sync.dma_start(t[:], wq[ct * 128:(ct + 1) * 128, :])
            wq_sb.append(t)
        # idn/wo/bo/g2 are needed only late; their dma_starts are emitted
        # after the projections so early consumers don't wait on them (DMA
        # completion sems are threshold-counted per queue).
        idn_sb = const.tile([128, 128], BF16, tag="idn")
        wo_sb = [const.tile([128, C], BF16, tag=f"wo{kt}", name=f"wo{kt}")
                 for kt in range(4)]
        bo_sb = [const.tile([128, 1], F32, tag=f"bo{mt}", name=f"bo{mt}")
                 for mt in range(2)]
        g2_sb = [const.tile([128, 1], F32, tag=f"g2{mt}", name=f"g2{mt}")
                 for mt in range(2)]

        def _late_dmas():
            # Pool SWDGE: keeps the SP HWDGE queue (x, wq, out) clean so
            # early consumers' coalesced DMA-clock waits release early.
            nc.gpsimd.dma_start(idn_sb[:], idn)
            for kt in range(4):
                nc.gpsimd.dma_start(wo_sb[kt][:],
                                    wo[kt * 128:(kt + 1) * 128, :])
            for mt in range(2):
                nc.gpsimd.dma_start(bo_sb[mt][:],
                                    bo[mt * 128:(mt + 1) * 128, :])
                nc.gpsimd.dma_start(g2_sb[mt][:],
                                    g2[mt * 128:(mt + 1) * 128, :])

        # ------------- unified psum pipeline: norm + proj + attention ------
        # ONE rotating psum arena (3 x [128,1024]) serves the norm's ss/bc,
        # all projections, and the sim chunks, so no phase ever blocks on a
        # psum-bank WAR against a different pool's slowest reader.
        xn_sb = [[None] * 2 for _ in range(NCT)]
        q_sb, k_sb = [None] * 4, [None] * 4
        vt_sb = [None] * NJ
        y4_sb = []
        with tc.tile_pool(name="ps", bufs=3, space="PSUM") as psp, \
                tc.tile_pool(name="ps_yt", bufs=2, space="PSUM") as ps_yt:

            def pstile(name):
                return psp.tile([128, 1024], F32, tag="sim", name=name)

            # --- input rms-norm: per-engine stage batching so the DVE
            # queue never buries a later half behind an earlier half ---
            s1 = widep.tile([1, L], F32R, tag="wide")
            sq8t, sst, stmpt, bct = [], [], [], []
            for half in range(2):
                hs = slice(half * 1024, (half + 1) * 1024)
                t = sqp.tile([128, 2, 1024], BF16, tag="sq8",
                             name=f"sq8_{half}")
                for ct in range(NCT):
                    nc.vector.tensor_mul(t[:, ct, :],
                                         x_sb[ct][:, hs], x_sb[ct][:, hs])
                sq8t.append(t)
            for half in range(2):
                sst.append(pstile(f"ssps{half}"))
                for n in range(2):
                    for ct in range(NCT):
                        nc.tensor.matmul(
                            sst[half][0:1, n * 512:(n + 1) * 512],
                            ones_col[:],
                            sq8t[half][:, ct, n * 512:(n + 1) * 512],
                            start=(ct == 0), stop=(ct == NCT - 1))
            for half in range(2):
                stmp = widep.tile([1, 1024], F32, tag="stmp",
                                  name=f"stmp{half}")
                nc.scalar.activation(stmp[:], sst[half][0:1, :], AF.Sqrt,
                                     bias=eps_t[:], scale=1.0 / C)
                stmpt.append(stmp)
            for half in range(2):
                hs = slice(half * 1024, (half + 1) * 1024)
                nc.vector.reciprocal(s1[0:1, hs], stmpt[half][:])
            for half in range(2):
                bct.append(pstile(f"bcps{half}"))
                for n in range(2):
                    cs = slice(half * 1024 + n * 512,
                               half * 1024 + (n + 1) * 512)
                    nc.tensor.matmul(bct[half][:, n * 512:(n + 1) * 512],
                                     ones_row[:], s1[0:1, cs],
                                     start=True, stop=True)
            for half in range(2):
                hs = slice(half * 1024, (half + 1) * 1024)
                for ct in range(NCT):
                    t = xnp.tile([128, 1024], BF16, tag="xn",
                                 name=f"xn{ct}_{half}")
                    nc.vector.tensor_mul(t[:], x_sb[ct][:, hs],
                                         bct[half][:, :])
                    xn_sb[ct][half] = t

            # --- projection helpers (psum from the shared arena) ---
            def make_q(mt):
                ps = pstile(f"qps{mt}")
                for n in range(LQ // 512):
                    for ct in range(NCT):
                        nc.tensor.matmul(
                            ps[:, n * 512:(n + 1) * 512],
                            wq_sb[ct][:, mt * 128:(mt + 1) * 128],
                            xn_sb[ct][0][:, n * 512:(n + 1) * 512],
                            start=(ct == 0), stop=(ct == NCT - 1))
                t = qp.tile([128, LQ], BF16, tag="q", name=f"qsb{mt}")
                nc.scalar.copy(t[:], ps[:, :])
                q_sb[mt] = t

            def make_k_half(mt, half, eng):
                if half == 0:
                    k_sb[mt] = kp.tile([128, L], BF16, tag="k",
                                       name=f"ksb{mt}")
                t = k_sb[mt]
                ps = pstile(f"kps{mt}_{half}")
                for n in range(2):
                    for ct in range(NCT):
                        nc.tensor.matmul(
                            ps[:, n * 512:(n + 1) * 512],
                            wq_sb[ct][:, HID + mt * 128:HID + (mt + 1) * 128],
                            xn_sb[ct][half][:, n * 512:(n + 1) * 512],
                            start=(ct == 0), stop=(ct == NCT - 1))
                if eng is nc.scalar:
                    eng.copy(t[:, half * LQ:(half + 1) * LQ], ps[:, :])
                else:
                    eng.tensor_copy(t[:, half * LQ:(half + 1) * LQ], ps[:, :])

            def make_vt(jt):
                ps = pstile(f"vps{jt}")
                for ct in range(NCT):
                    nc.tensor.matmul(ps[:, 0:512],
                                     xn_sb[ct][jt // 8][:, (jt % 8) * 128:
                                                        (jt % 8 + 1) * 128],
                                     wq_sb[ct][:, 2 * HID:3 * HID],
                                     start=(ct == 0), stop=(ct == NCT - 1))
                t = vtp.tile([128, H, D + 1], BF16, tag="vt", name=f"vt{jt}")
                src = ps[:, 0:512].rearrange("p (h e) -> p h e", e=D)
                if jt % 2 == 0:
                    nc.vector.tensor_copy(t[:, :, 0:D], src)
                else:
                    nc.scalar.copy(t[:, :, 0:D], src)
                nc.gpsimd.tensor_copy(
                    t[:, :, D:D + 1].rearrange("p h o -> p (h o)"),
                    stage[:, 0:H])
                vt_sb[jt] = t

            make_q(0)
            make_k_half(0, 0, nc.vector)
            make_k_half(0, 1, nc.scalar)
            _late_dmas()
            for jt in range(8):
                make_vt(jt)

            # --- attention: head h sims + exp stream while head h-1's yT
            # accumulates from its fully-materialized pt tile; projections
            # for later heads interleave into h0-h3's slots. ---
            pt_sb = [None] * H
            yts_sb = [None] * 4
            out_ps = []
            state = {}

            def sim_chunk(h, jt):
                mt, po = h // 2, (h % 2) * D
                sps = pstile(f"sps{h}_{jt}")
                for n in range(LQ // 512):
                    nc.tensor.matmul(
                        sps[:, n * 512:(n + 1) * 512],
                        k_sb[mt][po:po + D, jt * 128:(jt + 1) * 128],
                        q_sb[mt][po:po + D, n * 512:(n + 1) * 512],
                        start=True, stop=True)
                dst = pt_sb[h][:, jt, :]
                if jt in DVE_JT:
                    nc.vector.tensor_scalar(
                        dst.bitcast(U16), sps[:, :], SCH_A, SCH_B,
                        op0=MUL, op1=ADD)
                else:
                    nc.scalar.activation(dst, sps[:, :], AF.Exp,
                                         bias=zero_t[:], scale=0.125)

            def yt_start(h):
                state["yps"] = ps_yt.tile([128, 4, 128], F32, tag="yt",
                                          name=f"yt{h}_0")

            def yt_step(h, step):
                """steps 0..31: (half, ic, jts 0..15) in 8-instr groups.
                step s covers half=s//16, ic=(s//4)%4, jts 4*(s%4)..+4."""
                half, ic, j0 = step // 16, (step // 4) % 4, 4 * (step % 4)
                po = (h % 2) * D
                yps, pt = state["yps"], pt_sb[h]
                for jt in range(j0, j0 + 4):
                    nc.tensor.matmul(
                        yps[:, ic, 0:D + 1],
                        pt[:, jt, (half * 4 + ic) * 128:(half * 4 + ic + 1) * 128],
                        vt_sb[jt][:, h, :],
                        start=(jt == 0 and ic == 0),
                        stop=(jt == NJ - 1),
                        skip_group_check=True)
                if step % 16 == 15:
                    if half == 0:
                        inv = invp.tile([128, 8], F32, tag="inv",
                                        name=f"inv{h}")
                        state["inv"] = inv
                    else:
                        inv = state["inv"]
                    nc.vector.reciprocal(
                        inv[:, half * 4:(half + 1) * 4],
                        yps[:, :, D:D + 1].rearrange("p a b -> p (a b)"))
                    nc.vector.tensor_mul(
                        yts_sb[h // 2][:, half * 4:(half + 1) * 4, po:po + D],
                        yps[:, :, 0:D],
                        inv[:, half * 4:(half + 1) * 4].unsqueeze(2)
                        .broadcast_to((128, 4, D)))
                    if half == 0:
                        state["yps"] = ps_yt.tile([128, 4, 128], F32,
                                                  tag="yt", name=f"yt{h}_1")

            def transpose_pair(pair):
                rt = pstile(f"y4ps{pair}")
                y4ps = rt[:].bitcast(BF16)[:, 0:LQ]
                for ich in range(NI):
                    nc.tensor.transpose(y4ps[:, ich * 128:(ich + 1) * 128],
                                        yts_sb[pair][:, ich, :], idn_sb[:])
                y4 = y4p.tile([128, LQ], BF16, tag="y4", name=f"y4_{pair}")
                nc.vector.tensor_copy(y4[:], y4ps)
                y4_sb.append(y4)

            for h in range(H + 1):
                if h < H:
                    pt_sb[h] = ptp.tile([128, NJ, LQ], BF16, tag="pt",
                                        name=f"pt{h}")
                    if h % 2 == 0:
                        yts_sb[h // 2] = ytsp.tile([128, NI, 128], BF16,
                                                   tag="yts",
                                                   name=f"yts{h // 2}")
                if h > 0:
                    yt_start(h - 1)
                for jt in range(NJ):
                    if h < H:
                        sim_chunk(h, jt)
                    if h == 0 and jt >= 8:
                        make_vt(jt)
                    elif h in (1, 2, 3):
                        mt = h
                        if jt == 2:
                            make_q(mt)
                        elif jt == 6:
                            make_k_half(mt, 0,
                                        nc.scalar if mt % 2 == 1 else nc.vector)
                        elif jt == 10:
                            make_k_half(mt, 1,
                                        nc.vector if mt % 2 == 1 else nc.scalar)
                    if h > 0 and jt > 0:
                        yt_step(h - 1, 2 * jt - 2)
                        yt_step(h - 1, 2 * jt - 1)
                        if jt == NJ - 1:
                            yt_step(h - 1, 30)
                            yt_step(h - 1, 31)
                    if jt == 4 and h > 1 and h % 2 == 1:
                        transpose_pair(h // 2 - 1)
                    if h == H and jt in (2, 4):
                        mt = 0 if jt == 2 else 1
                        ops = pstile(f"ops{mt}")
                        out_ps.append(ops)
                        for pair in range(3):
                            for n in range(LQ // 512):
                                nc.tensor.matmul(
                                    ops[:, n * 512:(n + 1) * 512],
                                    wo_sb[pair][:, mt * 128:(mt + 1) * 128],
                                    y4_sb[pair][:, n * 512:(n + 1) * 512],
                                    start=(pair == 0), stop=False)
            transpose_pair(3)

            # ------- out-proj tail + bias + final rms-norm (in-ring) -------
            sq2 = sqp.tile([128, 2, LQ], BF16, tag="sq2")
            ss2r = pstile("ss2ps")
            out_sb = []
            for mt in range(2):
                for n in range(LQ // 512):
                    nc.tensor.matmul(
                        out_ps[mt][:, n * 512:(n + 1) * 512],
                        wo_sb[3][:, mt * 128:(mt + 1) * 128],
                        y4_sb[3][:, n * 512:(n + 1) * 512],
                        start=False, stop=True)
                t = outp.tile([128, LQ], BF16, tag="osb")
                nc.vector.tensor_scalar_add(t[:], out_ps[mt][:, :],
                                            bo_sb[mt][:])
                out_sb.append(t)
                nc.vector.tensor_mul(sq2[:, mt, :], t[:], t[:])
                for n in range(LQ // 512):
                    nc.tensor.matmul(ss2r[0:1, n * 512:(n + 1) * 512],
                                     ones_col[:],
                                     sq2[:, mt, n * 512:(n + 1) * 512],
                                     start=(mt == 0), stop=(mt == 1))
            s2tmp = widep.tile([1, LQ], F32, tag="stmp", name="s2tmp")
            nc.scalar.activation(s2tmp[:], ss2r[0:1, :], AF.Sqrt,
                                 bias=eps_t[:], scale=1.0 / C)
            s2 = widep.tile([1, LQ], F32R, tag="wide", name="s2")
            nc.vector.reciprocal(s2[:], s2tmp[:])
            bc2r = pstile("bc2ps")
            for n in range(LQ // 512):
                nc.tensor.matmul(bc2r[:, n * 512:(n + 1) * 512],
                                 ones_row[:], s2[0:1, n * 512:(n + 1) * 512],
                                 start=True, stop=True)
            for mt in range(2):
                t = finp.tile([128, LQ], F32, tag="fin")
                for half in range(2):
                    hs = slice(half * 512, (half + 1) * 512)
                    nc.vector.scalar_tensor_tensor(
                        t[:, hs], out_sb[mt][:, hs], g2_sb[mt][:],
                        bc2r[:, hs], op0=MUL, op1=MUL)
                    nc.sync.dma_start(out[mt * 128:(mt + 1) * 128, hs],
                                      t[:, hs])




_NC = None


def _get_nc():
    global _NC
    if _NC is None:
        nc = bacc.Bacc("TRN2", target_bir_lowering=False, debug=False,
                       enable_asserts=False, num_devices=8)
        x_d = nc.dram_tensor("x", [C, L], BF16, kind="ExternalInput")
        wq_d = nc.dram_tensor("wqkvT", [C, 3 * HID], BF16, kind="ExternalInput")
        wo_d = nc.dram_tensor("woutT", [HID, C], BF16, kind="ExternalInput")
        b_d = nc.dram_tensor("bout", [C, 1], F32, kind="ExternalInput")
        g2_d = nc.dram_tensor("g2v", [C, 1], F32, kind="ExternalInput")
        idn_d = nc.dram_tensor("idn", [128, 128], BF16, kind="ExternalInput")
        out_d = nc.dram_tensor("out", [C, LQ], F32, kind="ExternalOutput")
        with tile.TileContext(nc) as tc:
            _body(tc, x_d.ap(), wq_d.ap(), wo_d.ap(), b_d.ap(), g2_d.ap(),
                  idn_d.ap(), out_d.ap())
        nc.compile()
        _NC = nc
    return _NC


def _in_maps(x, g1, w_qkv, w_out, b_out, g2):
    BFH = ml_dtypes.bfloat16
    w2 = (np.asarray(w_qkv, np.float32)
          * np.asarray(g1, np.float32).reshape(1, C))
    wqkvT = np.ascontiguousarray(w2.T).astype(BFH)
    woutT = np.ascontiguousarray(np.asarray(w_out, np.float32).T).astype(BFH)
    bo = np.asarray(b_out, np.float32).reshape(C, 1)
    g2v = np.asarray(g2, np.float32).reshape(C, 1)
    idn = np.eye(128, dtype=BFH)
    maps = []
    for core in range(8):
        b, half = divmod(core, 2)
        xb = np.asarray(x[b], np.float32)
        x_core = np.ascontiguousarray(np.concatenate(
            [xb[:, half * LQ:(half + 1) * LQ],
             xb[:, (1 - half) * LQ:(2 - half) * LQ]], axis=1)).astype(BFH)
        maps.append({"x": x_core, "wqkvT": wqkvT, "woutT": woutT,
                     "bout": bo, "g2v": g2v, "idn": idn})
    return maps


def _assemble(results):
    out = np.empty((B, C, L), np.float32)
    for core in range(8):
        b, half = divmod(core, 2)
        out[b][:, half * LQ:(half + 1) * LQ] = results[core]["out"]
    return out


def kernel(x, g1, w_qkv, w_out, b_out, g2, _trace=False, _tmpdir=None):
    res = run_bass_kernel_spmd(_get_nc(),
                               _in_maps(x, g1, w_qkv, w_out, b_out, g2),
                               core_ids=list(range(8)), trace=_trace,
                               tmpdir=_tmpdir)
    out = _assemble(res.results)
    if _trace:
        return out, res
    return out
